# revision 1
# baseline (speedup 1.0000x reference)
"""Trainium2 Bass kernel for RoPE causal multi-head attention.

Stall-minimized software-pipelined Trainium2 Bass kernel for RoPE
causal MHA.

Microbenchmarks showed per-instruction HW costs are ~1.2x the cost model
across engines; the attention gap (2.06x) is dependency-stall latency.
v4 attacks stalls:
  - ONE psum tile shape [128, 2, 512] (2 banks) everywhere: scores get
    bufs=3 rotation depth (tag "big", shared with QKV/rope/v/Wo packing),
    av accumulators tag "av" bufs=1.  3-deep score rotation lets the PE
    run up to 3 pair-tiles ahead of the exp consumer.
  - av matmuls trail score/exp emission by S=2 lk tiles (PE never parks
    on a pending exp).
  - QKV q/k chains pack t=0/1 into the two banks of one tile (16 matmuls
    per dependency quantum); v packs 2 lt tiles; Wo packs 2 m tiles with
    a merged [128, 2, 512] PSUM->SBUF copy; rope packs t=0/1 with a
    merged add.  Fewer, bigger dependency edges and DVE instructions.
  - QKV(n+1) + Wo(n-1) units dealt round-robin into attention(n) slots.
"""

import numpy as np

import concourse.bass as bass
import concourse.mybir as mybir
from concourse import bacc
import concourse.tile as tile
from concourse.bass_utils import run_bass_kernel_spmd

F32 = mybir.dt.float32
F32R = mybir.dt.float32r
EXP = mybir.ActivationFunctionType.Exp

B, L, D, NH, HD = 2, 2048, 1024, 16, 64
HPC = NH // 4
DQ = HPC * HD
NB = L // 512
NT = L // 128
S = 2


def build_kernel(mask_mode: str, repeat: int = 1) -> bass.Bass:
    nc = bacc.Bacc(None)
    xT = nc.declare_dram_parameter("xT", [NB, 128, 8, 512], F32R, isOutput=False)
    wqT = nc.declare_dram_parameter("wqT", [128, 8, DQ], F32R, isOutput=False)
    wkT = nc.declare_dram_parameter("wkT", [128, 8, DQ], F32R, isOutput=False)
    wvT = nc.declare_dram_parameter("wvT", [128, 8, DQ], F32R, isOutput=False)
    woT = nc.declare_dram_parameter("woT", [128, 2, D], F32R, isOutput=False)
    cosT = nc.declare_dram_parameter("cosT", [128, L], F32, isOutput=False)
    sinT = nc.declare_dram_parameter("sinT", [128, L], F32, isOutput=False)
    rT = nc.declare_dram_parameter("rT", [128, 128], F32R, isOutput=False)
    tri = nc.declare_dram_parameter("tri", [128, 128], F32R, isOutput=False)
    ones = nc.declare_dram_parameter("ones", [128, 64], F32R, isOutput=False)
    if mask_mode == "general":
        emT = nc.declare_dram_parameter("emT", [L, L], F32, isOutput=False)
    outT = nc.declare_dram_parameter("outT", [D, L], F32, isOutput=True)

    with tile.TileContext(nc) as tc:
        with (
            tc.tile_pool(name="const", bufs=1) as const,
            tc.tile_pool(name="persist", bufs=1) as persist,
            tc.tile_pool(name="psp", bufs=1, space="PSUM") as ps,
            tc.tile_pool(name="xs", bufs=2) as xs,
            tc.tile_pool(name="wp", bufs=1) as wp,
            tc.tile_pool(name="qs", bufs=2) as qs_pool,
            tc.tile_pool(name="ep", bufs=4) as ep,
            tc.tile_pool(name="em", bufs=2) as emp,
            tc.tile_pool(name="rp", bufs=2) as rp,
            tc.tile_pool(name="oc", bufs=3) as ocp,
        ):
            cos_sb = const.tile([128, L], F32)
            sin_sb = const.tile([128, L], F32)
            rT_sb = const.tile([128, 128], F32R)
            tri_sb = const.tile([128, 128], F32R)
            wo_sb = const.tile([128, 2, D], F32R)

            q_sb = persist.tile([128, 2, L], F32R)
            k_sb = persist.tile([128, 2, L], F32R)
            v_sb = persist.tile([128, NT, HPC, HD + 1], F32R)
            o_sb = persist.tile([128, 2, L], F32R)

            wq_sb = wp.tile([128, 8, DQ], F32R)
            wk_sb = wp.tile([128, 8, DQ], F32R)
            wv_sb = wp.tile([128, 8, DQ], F32R)

            def qk_unit(xt, i, w_sb, dst, qs_blk, sl):
                """Both t-halves of a q or k projection block: 16 matmuls
                into one 2-bank psum, then per-half cos/sin muls."""
                p = ps.tile([128, 2, 512], F32, tag="big", bufs=3, name="qkp")
                for t in range(2):
                    for kc in range(8):
                        nc.tensor.matmul(
                            p[:, t],
                            w_sb[:, kc, t * 128 : (t + 1) * 128],
                            xt[:, kc, :],
                            start=(kc == 0),
                            stop=(kc == 7),
                        )
                for t in range(2):
                    nc.vector.tensor_mul(out=dst[:, t, sl], in0=p[:, t], in1=cos_sb[:, sl])
                    nc.vector.tensor_mul(out=qs_blk[:, 2 * i + t], in0=p[:, t], in1=sin_sb[:, sl])

            def v_unit(xt, n, j):
                """Two lt tiles of the v projection into one 2-bank psum,
                merged copy into v_sb."""
                lt0 = 4 * n + 2 * j
                p = ps.tile([128, 2, 512], F32, tag="big", bufs=3, name="vp")
                for a in range(2):
                    for kc in range(8):
                        nc.tensor.matmul(
                            p[:, a, :DQ],
                            xt[:, kc, (2 * j + a) * 128 : (2 * j + a + 1) * 128],
                            wv_sb[:, kc, :],
                            start=(kc == 0),
                            stop=(kc == 7),
                        )
                nc.vector.tensor_copy(
                    v_sb[:, lt0 : lt0 + 2, :, 0:HD],
                    p[:, :, :DQ].rearrange("p a (h e) -> p a h e", h=HPC),
                )

            def rope_unit(i, dst, qs_blk, sl):
                """R@(p*sin) for both t-halves, merged add into dst."""
                p = ps.tile([128, 2, 512], F32, tag="big", bufs=3, name="rp")
                for t in range(2):
                    nc.tensor.matmul(p[:, t], rT_sb[:], qs_blk[:, 2 * i + t],
                                     start=True, stop=True)
                nc.vector.tensor_add(
                    out=dst[:, :, sl], in0=dst[:, :, sl], in1=p[:]
                )

            def qkv_units(n, xt):
                sl = slice(n * 512, (n + 1) * 512)
                qs_blk = qs_pool.tile([128, 4, 512], F32R, tag="qs", name="qs_blk")
                units = [
                    lambda: qk_unit(xt, 0, wq_sb, q_sb, qs_blk, sl),
                    lambda: qk_unit(xt, 1, wk_sb, k_sb, qs_blk, sl),
                    lambda: v_unit(xt, n, 0),
                    lambda: v_unit(xt, n, 1),
                    lambda: rope_unit(0, q_sb, qs_blk, sl),
                    lambda: rope_unit(1, k_sb, qs_blk, sl),
                ]
                return units

            def wo_unit(n, j):
                """Two m tiles of the output projection: 4 matmuls into one
                2-bank psum, merged copy, 2 output DMAs."""
                sl = slice(n * 512, (n + 1) * 512)
                p = ps.tile([128, 2, 512], F32, tag="big", bufs=3, name="wop")
                for a in range(2):
                    m = 2 * j + a
                    for kc in range(2):
                        nc.tensor.matmul(
                            p[:, a],
                            wo_sb[:, kc, m * 128 : (m + 1) * 128],
                            o_sb[:, kc, sl],
                            start=(kc == 0),
                            stop=(kc == 1),
                        )
                oc = ocp.tile([128, 2, 512], F32, tag="oc", name="oc")
                nc.vector.tensor_copy(oc[:], p[:])
                for a in range(2):
                    m = 2 * j + a
                    nc.sync.dma_start(outT[m * 128 : (m + 1) * 128, sl], oc[:, a])

            def wo_units(n):
                return [(lambda j=j: wo_unit(n, j)) for j in range(4)]

            def attention(n, filler):
                sl = slice(n * 512, (n + 1) * 512)
                for hp in range(2):
                    heads = (2 * hp, 2 * hp + 1)
                    t = hp
                    if mask_mode == "causal":
                        lk_tiles = list(range(0, 4 * n + 4))
                    else:
                        lk_tiles = list(range(NT))
                    T = len(lk_tiles)
                    av2 = ps.tile([128, 2, 512], F32, tag="av", bufs=1,
                                  name=f"av_{n}_{hp}")
                    e2s = {}
                    for idx in range(T + S):
                        if idx < T:
                            lk = lk_tiles[idx]
                            boundary = mask_mode == "causal" and lk >= 4 * n
                            c0 = 128 * (lk - 4 * n) if boundary else 0
                            p2 = ps.tile([128, 2, 512], F32, tag="big", bufs=3,
                                         name=f"sc_{n}_{lk}_{hp}")
                            for h in heads:
                                ro = 64 * (h % 2)
                                nc.tensor.matmul(
                                    p2[:, h % 2, c0:],
                                    k_sb[ro : ro + 64, t, lk * 128 : (lk + 1) * 128],
                                    q_sb[ro : ro + 64, t, n * 512 + c0 : (n + 1) * 512],
                                    start=True,
                                    stop=True,
                                    tile_position=(ro, 0),
                                )
                            e2 = ep.tile([128, 2, 512], F32R, tag="e",
                                         name=f"e_{n}_{lk}_{hp}")
                            nc.scalar.activation(e2[:, :, c0:], p2[:, :, c0:], EXP,
                                                 scale=0.125)
                            if boundary:
                                for h in heads:
                                    nc.gpsimd.tensor_mul(
                                        out=e2[:, h % 2, c0 : c0 + 128],
                                        in0=e2[:, h % 2, c0 : c0 + 128],
                                        in1=tri_sb[:],
                                    )
                            elif mask_mode == "general":
                                em = emp.tile([128, 512], F32, tag="em", name="em")
                                nc.sync.dma_start(
                                    em[:],
                                    emT[lk * 128 : (lk + 1) * 128, n * 512 : (n + 1) * 512],
                                )
                                for h in heads:
                                    nc.vector.tensor_mul(
                                        out=e2[:, h % 2], in0=e2[:, h % 2], in1=em[:]
                                    )
                            e2s[idx] = e2
                        if idx >= S:
                            j = idx - S
                            lk = lk_tiles[j]
                            boundary = mask_mode == "causal" and lk >= 4 * n
                            c0 = 128 * (lk - 4 * n) if boundary else 0
                            e2 = e2s.pop(j)
                            for h in heads:
                                nc.tensor.matmul(
                                    av2[0 : HD + 1, h % 2, c0:],
                                    v_sb[:, lk, h, :],
                                    e2[:, h % 2, c0:],
                                    start=(j == 0),
                                    stop=(j == T - 1),
                                )
                        filler()
                    rec2 = rp.tile([1, 2, 512], F32, tag="rec", name="rec")
                    nc.vector.reciprocal(rec2[:], av2[HD : HD + 1, :, :])
                    rbc2 = rp.tile([64, 2, 512], F32, tag="rbc", name="rbc")
                    nc.gpsimd.partition_broadcast(rbc2[:], rec2[:])
                    for h in heads:
                        ro = 64 * (h % 2)
                        nc.vector.tensor_mul(
                            out=o_sb[ro : ro + 64, t, sl],
                            in0=av2[0:HD, h % 2, :],
                            in1=rbc2[:, h % 2, :],
                        )

            def _emit_body():
                xt0 = xs.tile([128, 8, 512], F32R, tag="xt", name="xt0")
                for kc in range(8):
                    nc.sync.dma_start(wq_sb[:, kc], wqT[:, kc])
                    nc.sync.dma_start(xt0[:, kc], xT[0:1, :, kc, :].rearrange("a p f -> p (a f)"))
                nc.sync.dma_start(wk_sb[:], wkT[:])
                nc.sync.dma_start(wv_sb[:], wvT[:])
                nc.sync.dma_start(cos_sb[:, 0:512], cosT[:, 0:512])
                nc.sync.dma_start(sin_sb[:, 0:512], sinT[:, 0:512])
                nc.sync.dma_start(rT_sb[:], rT[:])
                nc.sync.dma_start(tri_sb[:], tri[:])
                ones_sb = const.tile([128, 64], F32R, name="ones_sb")
                nc.sync.dma_start(ones_sb[:], ones[:])
                nc.vector.tensor_copy(
                    v_sb[:, :, :, HD : HD + 1],
                    ones_sb.rearrange("p (a b c) -> p a b c", a=NT, b=HPC, c=1),
                )
                for c in range(1, 4):
                    nc.sync.dma_start(cos_sb[:, c * 512 : (c + 1) * 512],
                                      cosT[:, c * 512 : (c + 1) * 512])
                    nc.sync.dma_start(sin_sb[:, c * 512 : (c + 1) * 512],
                                      sinT[:, c * 512 : (c + 1) * 512])
                nc.sync.dma_start(wo_sb[:], woT[:])

                for u in qkv_units(0, xt0):
                    u()

                for n in range(NB):
                    pending = []
                    if n + 1 < NB:
                        xt = xs.tile([128, 8, 512], F32R, tag="xt", name=f"xt{n+1}")
                        nc.sync.dma_start(
                            xt[:], xT[n + 1 : n + 2].rearrange("a p o f -> p o (a f)")
                        )
                        pending += qkv_units(n + 1, xt)
                    if n - 1 >= 0:
                        pending += wo_units(n - 1)
                    nslots = 2 * ((4 * n + 4 if mask_mode == "causal" else NT) + S)
                    pending_init = list(pending)
                    emitted = [0]

                    def filler(pending=pending, emitted=emitted,
                               pending_init=pending_init, nslots=nslots):
                        emitted[0] += 1
                        while pending and emitted[0] * len(pending_init) >= (
                            len(pending_init) - len(pending) + 1
                        ) * nslots:
                            pending.pop(0)()

                    attention(n, filler)
                    for u in pending:
                        u()
                for u in wo_units(NB - 1):
                    u()

            if repeat == 1:
                _emit_body()
            else:
                with tc.For_i(0, repeat, 1):
                    _emit_body()
    nc.finalize()
    return nc


_compiled = {}


def _get_kernel(mask_mode):
    if mask_mode not in _compiled:
        _compiled[mask_mode] = build_kernel(mask_mode)
    return _compiled[mask_mode]


def kernel(x, freqs, attention_mask, Wq, Wk, Wv, Wo, _trace=False, _trace_kwargs=None):
    x = np.asarray(x, dtype=np.float32)
    freqs = np.asarray(freqs, dtype=np.float32)
    mask = np.asarray(attention_mask, dtype=np.float32).reshape(L, L)
    Wq, Wk, Wv, Wo = (np.asarray(w, dtype=np.float32) for w in (Wq, Wk, Wv, Wo))

    causal_ref = np.where(np.tri(L, dtype=bool), 0.0, -1e9).astype(np.float32)
    if not mask.any():
        mask_mode = "zero"
    elif np.array_equal(mask, causal_ref):
        mask_mode = "causal"
    else:
        mask_mode = "general"

    fr, fi = freqs[..., 0], freqs[..., 1]
    cosE = np.repeat(fr, 2, axis=1).T
    sinE = np.repeat(fi, 2, axis=1).T
    cos128 = np.ascontiguousarray(np.concatenate([cosE, cosE], axis=0))
    sin128 = np.ascontiguousarray(np.concatenate([sinE, sinE], axis=0))
    R = np.zeros((128, 128), dtype=np.float32)
    for i in range(64):
        R[2 * i, 2 * i + 1] = -1.0
        R[2 * i + 1, 2 * i] = 1.0
    rT = np.ascontiguousarray(R.T)
    triM = (np.arange(128)[:, None] <= np.arange(128)[None, :]).astype(np.float32)

    in_maps = []
    for c in range(8):
        b, g = divmod(c, 4)
        rows = slice(DQ * g, DQ * (g + 1))
        def tile_w(wt):
            return np.ascontiguousarray(wt.reshape(8, 128, -1).transpose(1, 0, 2))
        xt_full = x[b].T
        xt4 = np.ascontiguousarray(
            xt_full.reshape(8, 128, NB, 512).transpose(2, 1, 0, 3)
        )
        wot = Wo[:, rows].T
        m = {
            "xT": xt4,
            "wqT": tile_w(Wq[rows].T),
            "wkT": tile_w(Wk[rows].T),
            "wvT": tile_w(Wv[rows].T),
            "woT": np.ascontiguousarray(wot.reshape(2, 128, D).transpose(1, 0, 2)),
            "cosT": cos128,
            "sinT": sin128,
            "rT": rT,
            "tri": triM,
            "ones": np.ones((128, 64), dtype=np.float32),
        }
        if mask_mode == "general":
            m["emT"] = np.ascontiguousarray(np.exp(mask).T)
        in_maps.append(m)

    nc = _get_kernel(mask_mode)
    kw = {}
    if _trace:
        kw = dict(trace=True, trace_kwargs=_trace_kwargs or {})
    res = run_bass_kernel_spmd(nc, in_maps, list(range(8)), **kw)
    out = np.empty((B, L, D), dtype=np.float32)
    for b in range(B):
        acc = res.results[4 * b]["outT"].astype(np.float32)
        for g in range(1, 4):
            acc = acc + res.results[4 * b + g]["outT"]
        out[b] = acc.T
    kernel.last_result = res
    return out



# revision 17
# speedup vs baseline: 1.2296x; 1.2296x over previous
"""Trainium2 Bass kernel for RoPE causal multi-head attention (v5).

v5 changes vs v4 (307us baseline):
  - bf16 compute: x/W/q/k/v/e/o SBUF tensors and all matmul operands in
    bfloat16 (PSUM stays fp32, exp reads PSUM fp32).  Halves DMA traffic
    (incl. the serial head/tail), halves SBUF bandwidth, and enables fast
    weight load (FWL) on HW.  cos/sin stay fp32 (DVE reads PSUM fp32
    anyway, so no speed cost; better rope accuracy).
  - Output DMA'd as bf16, host upcasts+reduces in fp32.
  - av PSUM double-buffered (bufs=2) so next hp's AV matmuls don't wait
    on the previous hp's rec/broadcast/normalize tail; score ring 2-deep.
  - ACT-load-aware filler dealing: exp is the per-block bottleneck for
    late blocks (exp elems grow ~8(n+1)us per block while attention PE
    work is only ~645ns/tile vs ~920ns/tile exp).  Wo fillers are
    deferred to block 3 (wo(0)+wo(1) into hp0, wo(2) into hp1) where PE
    would otherwise idle behind the scalar engine.
  - qk projection units split per t-half (finer deal granularity).
  - Per-bank reciprocal/broadcast/normalize chains; v-copy on gpsimd.
  - Merged output DMA (one per wo unit).
"""

import numpy as np
import ml_dtypes

import concourse.bass as bass
import concourse.mybir as mybir
from concourse import bacc
import concourse.tile as tile
from concourse.bass_utils import run_bass_kernel_spmd

F32 = mybir.dt.float32
BF16 = mybir.dt.bfloat16
EXP = mybir.ActivationFunctionType.Exp
COPY = mybir.ActivationFunctionType.Copy

B, L, D, NH, HD = 2, 2048, 1024, 16, 64
HPC = NH // 4
DQ = HPC * HD
NB = L // 512
NT = L // 128
S = 2
BIG_BUFS = 2
AV_BUFS = 2


PHASE_MARKS = []


def build_kernel(mask_mode: str, repeat: int = 1) -> bass.Bass:
    PHASE_MARKS.clear()
    nc = bacc.Bacc(None)

    def mark(label):
        PHASE_MARKS.append((label, nc.next_id()))
    xT = nc.declare_dram_parameter("xT", [NB, 128, 8, 512], BF16, isOutput=False)
    wqT = nc.declare_dram_parameter("wqT", [128, 8, DQ], BF16, isOutput=False)
    wkT = nc.declare_dram_parameter("wkT", [128, 8, DQ], BF16, isOutput=False)
    wvT = nc.declare_dram_parameter("wvT", [128, 8, DQ], BF16, isOutput=False)
    woT = nc.declare_dram_parameter("woT", [128, 2, D], BF16, isOutput=False)
    cosT = nc.declare_dram_parameter("cosT", [128, L], F32, isOutput=False)
    sinT = nc.declare_dram_parameter("sinT", [128, L], F32, isOutput=False)
    rT = nc.declare_dram_parameter("rT", [128, 128], BF16, isOutput=False)
    tri = nc.declare_dram_parameter("tri", [128, 128], BF16, isOutput=False)
    ones = nc.declare_dram_parameter("ones", [128, 64], BF16, isOutput=False)
    if mask_mode == "general":
        emT = nc.declare_dram_parameter("emT", [L, L], F32, isOutput=False)
    outT = nc.declare_dram_parameter("outT", [D, L], BF16, isOutput=True)

    with tile.TileContext(nc) as tc:
        with (
            tc.tile_pool(name="const", bufs=1) as const,
            tc.tile_pool(name="persist", bufs=1) as persist,
            tc.tile_pool(name="psp", bufs=1, space="PSUM") as ps,
            tc.tile_pool(name="xs", bufs=2) as xs,
            tc.tile_pool(name="wp", bufs=1) as wp,
            tc.tile_pool(name="qs", bufs=2) as qs_pool,
            tc.tile_pool(name="ep", bufs=EP_BUFS) as ep,
            tc.tile_pool(name="em", bufs=2) as emp,
            tc.tile_pool(name="rp", bufs=2) as rp,
            tc.tile_pool(name="oc", bufs=3) as ocp,
        ):
            cos_sb = const.tile([128, L], F32)
            sin_sb = const.tile([128, L], F32)
            rT_sb = const.tile([128, 128], BF16)
            tri_sb = const.tile([128, 128], BF16)
            wo_sb = const.tile([128, 2, D], BF16)

            q_sb = persist.tile([128, 2, L], BF16)
            k_sb = persist.tile([128, 2, L], BF16)
            v_sb = persist.tile([128, NT, HPC, HD + 1], BF16)
            o_sb = persist.tile([128, 2, L], BF16)

            wq_sb = wp.tile([128, 8, DQ], BF16)
            wk_sb = wp.tile([128, 8, DQ], BF16)
            wv_sb = wp.tile([128, 8, DQ], BF16)

            def qk_half(xt, i, t, w_sb, dst, qs_blk, sl, p_ref):
                """One t-half of a q or k projection block: 8 matmuls into
                bank t of a shared 2-bank psum, then cos/sin muls."""
                if t == 0:
                    p_ref[0] = ps.tile([128, 2, 512], F32, tag="big", bufs=BIG_BUFS,
                                       name="qkp")
                p = p_ref[0]
                for kc in range(8):
                    nc.tensor.matmul(
                        p[:, t],
                        w_sb[:, kc, t * 128 : (t + 1) * 128],
                        xt[:, kc, :],
                        start=(kc == 0),
                        stop=(kc == 7),
                    )
                nc.vector.tensor_mul(out=dst[:, t, sl], in0=p[:, t], in1=cos_sb[:, sl])
                nc.vector.tensor_mul(out=qs_blk[:, 2 * i + t], in0=p[:, t], in1=sin_sb[:, sl])

            def v_unit(xt, n, j):
                """Two lt tiles of the v projection into one 2-bank psum,
                merged copy into v_sb."""
                lt0 = 4 * n + 2 * j
                p = ps.tile([128, 2, 512], F32, tag="big", bufs=BIG_BUFS, name="vp")
                for a in range(2):
                    for kc in range(8):
                        nc.tensor.matmul(
                            p[:, a, :DQ],
                            xt[:, kc, (2 * j + a) * 128 : (2 * j + a + 1) * 128],
                            wv_sb[:, kc, :],
                            start=(kc == 0),
                            stop=(kc == 7),
                        )
                nc.vector.tensor_copy(
                    v_sb[:, lt0 : lt0 + 2, :, 0:HD],
                    p[:, :, :DQ].rearrange("p a (h e) -> p a h e", h=HPC),
                )

            def rope_unit(i, dst, qs_blk, sl):
                """R@(p*sin) for both t-halves, merged add into dst."""
                p = ps.tile([128, 2, 512], F32, tag="big", bufs=BIG_BUFS, name="rp")
                for t in range(2):
                    nc.tensor.matmul(p[:, t], rT_sb[:], qs_blk[:, 2 * i + t],
                                     start=True, stop=True)
                nc.vector.tensor_add(
                    out=dst[:, :, sl], in0=dst[:, :, sl], in1=p[:]
                )

            def qkv_units(n, xt):
                sl = slice(n * 512, (n + 1) * 512)
                qs_blk = qs_pool.tile([128, 4, 512], BF16, tag="qs", name="qs_blk")
                pq, pk = [None], [None]
                # rope units trail their qk halves by one unit so the PE
                # never waits on the DVE sin-mul chain
                units = [
                    lambda: qk_half(xt, 0, 0, wq_sb, q_sb, qs_blk, sl, pq),
                    lambda: qk_half(xt, 0, 1, wq_sb, q_sb, qs_blk, sl, pq),
                    lambda: qk_half(xt, 1, 0, wk_sb, k_sb, qs_blk, sl, pk),
                    lambda: rope_unit(0, q_sb, qs_blk, sl),
                    lambda: qk_half(xt, 1, 1, wk_sb, k_sb, qs_blk, sl, pk),
                    lambda: v_unit(xt, n, 0),
                    lambda: rope_unit(1, k_sb, qs_blk, sl),
                    lambda: v_unit(xt, n, 1),
                ]
                return units

            def wo_unit(n, j, on_act=False):
                """Two m tiles of the output projection: 4 matmuls into one
                2-bank psum, merged copy, one merged output DMA.  The copy
                can go on ACT (same act table as exp, so no reload) to
                parallelize the kernel tail."""
                sl = slice(n * 512, (n + 1) * 512)
                p = ps.tile([128, 2, 512], F32, tag="big", bufs=BIG_BUFS, name="wop")
                for a in range(2):
                    m = 2 * j + a
                    for kc in range(2):
                        nc.tensor.matmul(
                            p[:, a],
                            wo_sb[:, kc, m * 128 : (m + 1) * 128],
                            o_sb[:, kc, sl],
                            start=(kc == 0),
                            stop=(kc == 1),
                        )
                oc = ocp.tile([128, 2, 512], BF16, tag="oc", name="oc")
                if on_act:
                    nc.scalar.activation(oc[:], p[:], COPY, scale=1.0)
                else:
                    nc.vector.tensor_copy(oc[:], p[:])
                nc.sync.dma_start(
                    outT[256 * j : 256 * (j + 1), sl].rearrange(
                        "(a q) f -> q a f", a=2
                    ),
                    oc[:],
                )

            def wo_units(n, alt_act=False):
                return [(lambda j=j: wo_unit(n, j, on_act=alt_act and j % 2 == 0))
                        for j in range(4)]

            def attention_hp(n, hp, fillers):
                """One head-pair of attention block n, dealing `fillers`
                (list of unit thunks) evenly into the slot stream."""
                sl = slice(n * 512, (n + 1) * 512)
                heads = (2 * hp, 2 * hp + 1)
                t = hp
                if mask_mode == "causal":
                    lk_tiles = list(range(0, 4 * n + 4))
                else:
                    lk_tiles = list(range(NT))
                T = len(lk_tiles)
                mark(f"attn{n}.{hp}")
                nslots = T + S
                pending = list(fillers)
                ntot = len(pending)
                # start the pacing S slots ahead: the first AV can't issue
                # until exp(0) lands, so fillers should cover that bubble
                emitted = [S]

                def filler():
                    emitted[0] += 1
                    if not ntot:
                        return
                    while pending and emitted[0] * ntot >= (
                        ntot - len(pending) + 1
                    ) * nslots:
                        pending.pop(0)()

                av2 = ps.tile([128, 2, 512], F32, tag="av", bufs=AV_BUFS,
                              name=f"av_{n}_{hp}")
                e2s = {}
                for idx in range(T + S):
                    if idx < T:
                        lk = lk_tiles[idx]
                        boundary = mask_mode == "causal" and lk >= 4 * n
                        c0 = 128 * (lk - 4 * n) if boundary else 0
                        p2 = ps.tile([128, 2, 512], F32, tag="big", bufs=BIG_BUFS,
                                     name=f"sc_{n}_{lk}_{hp}")
                        for h in heads:
                            ro = 64 * (h % 2)
                            nc.tensor.matmul(
                                p2[:, h % 2, c0:],
                                k_sb[ro : ro + 64, t, lk * 128 : (lk + 1) * 128],
                                q_sb[ro : ro + 64, t, n * 512 + c0 : (n + 1) * 512],
                                start=True,
                                stop=True,
                                tile_position=(ro, 0),
                            )
                        e2 = ep.tile([128, 2, 512], BF16, tag="e",
                                     name=f"e_{n}_{lk}_{hp}")
                        nc.scalar.activation(e2[:, :, c0:], p2[:, :, c0:], EXP,
                                             scale=0.125)
                        if boundary:
                            for h in heads:
                                nc.gpsimd.tensor_mul(
                                    out=e2[:, h % 2, c0 : c0 + 128],
                                    in0=e2[:, h % 2, c0 : c0 + 128],
                                    in1=tri_sb[:],
                                )
                        elif mask_mode == "general":
                            em = emp.tile([128, 512], F32, tag="em", name="em")
                            nc.sync.dma_start(
                                em[:],
                                emT[lk * 128 : (lk + 1) * 128, n * 512 : (n + 1) * 512],
                            )
                            for h in heads:
                                nc.vector.tensor_mul(
                                    out=e2[:, h % 2], in0=e2[:, h % 2], in1=em[:]
                                )
                        e2s[idx] = e2
                    if idx >= S:
                        j = idx - S
                        lk = lk_tiles[j]
                        boundary = mask_mode == "causal" and lk >= 4 * n
                        c0 = 128 * (lk - 4 * n) if boundary else 0
                        e2 = e2s.pop(j)
                        for h in heads:
                            nc.tensor.matmul(
                                av2[0 : HD + 1, h % 2, c0:],
                                v_sb[:, lk, h, :],
                                e2[:, h % 2, c0:],
                                start=(j == 0),
                                stop=(j == T - 1),
                            )
                    filler()
                for u in pending:
                    u()
                # Normalize: per-bank reciprocals first, then broadcasts
                # (Pool) overlapping the muls (DVE) pipeline-style.
                rec2 = rp.tile([1, 2, 512], F32, tag="rec", name="rec")
                rbc2 = rp.tile([64, 2, 512], F32, tag="rbc", name="rbc")
                for hb in range(2):
                    nc.vector.reciprocal(rec2[:, hb], av2[HD : HD + 1, hb, :])
                    nc.gpsimd.partition_broadcast(rbc2[:, hb], rec2[:, hb])
                for hb in range(2):
                    ro = 64 * hb
                    nc.vector.tensor_mul(
                        out=o_sb[ro : ro + 64, t, sl],
                        in0=av2[0:HD, hb, :],
                        in1=rbc2[:, hb, :],
                    )

            def _emit_body():
                xt0 = xs.tile([128, 8, 512], BF16, tag="xt", name="xt0")
                # Startup DMAs spread across engine queues so the critical
                # first inputs (wq, x block 0, cos/sin head) land ASAP
                # instead of serializing behind one SP queue.
                nc.sync.dma_start(wq_sb[:, 0:4], wqT[:, 0:4])
                nc.scalar.dma_start(wq_sb[:, 4:8], wqT[:, 4:8])
                nc.sync.dma_start(
                    xt0[:, 0:4], xT[0:1, :, 0:4, :].rearrange("a p o f -> p o (a f)")
                )
                nc.scalar.dma_start(
                    xt0[:, 4:8], xT[0:1, :, 4:8, :].rearrange("a p o f -> p o (a f)")
                )
                nc.scalar.dma_start(cos_sb[:, 0:512], cosT[:, 0:512])
                nc.scalar.dma_start(sin_sb[:, 0:512], sinT[:, 0:512])
                nc.gpsimd.dma_start(wk_sb[:], wkT[:])
                nc.gpsimd.dma_start(rT_sb[:], rT[:])
                nc.gpsimd.dma_start(wv_sb[:], wvT[:])
                nc.gpsimd.dma_start(tri_sb[:], tri[:])
                ones_sb = const.tile([128, 64], BF16, name="ones_sb")
                nc.gpsimd.dma_start(ones_sb[:], ones[:])
                nc.vector.tensor_copy(
                    v_sb[:, :, :, HD : HD + 1],
                    ones_sb.rearrange("p (a b c) -> p a b c", a=NT, b=HPC, c=1),
                )
                nc.scalar.dma_start(cos_sb[:, 512:L], cosT[:, 512:L])
                nc.scalar.dma_start(sin_sb[:, 512:L], sinT[:, 512:L])
                nc.sync.dma_start(wo_sb[:], woT[:])

                mark("qkv0")
                for u in qkv_units(0, xt0):
                    u()

                # Per-(block, hp) filler assignment: qkv(n+1) feeds block n
                # (needed before attention(n+1)); Wo work is deferred to the
                # scalar-engine-bound final block.
                for n in range(NB):
                    qkv_pending = []
                    if n + 1 < NB:
                        xt = xs.tile([128, 8, 512], BF16, tag="xt", name=f"xt{n+1}")
                        nc.sync.dma_start(
                            xt[:], xT[n + 1 : n + 2].rearrange("a p o f -> p o (a f)")
                        )
                        qkv_pending = qkv_units(n + 1, xt)
                    if n < NB - 1:
                        nq = len(qkv_pending)
                        attention_hp(n, 0, qkv_pending[: (nq + 1) // 2])
                        attention_hp(n, 1, qkv_pending[(nq + 1) // 2 :])
                    else:
                        attention_hp(n, 0, wo_units(0) + wo_units(1))
                        attention_hp(n, 1, wo_units(2))
                mark("wo_tail")
                for u in wo_units(NB - 1, alt_act=True):
                    u()
                mark("end")

            if repeat == 1:
                _emit_body()
            else:
                with tc.For_i(0, repeat, 1):
                    _emit_body()
    nc.finalize()
    return nc


_compiled = {}


def _get_kernel(mask_mode):
    if mask_mode not in _compiled:
        _compiled[mask_mode] = build_kernel(mask_mode)
    return _compiled[mask_mode]


def kernel(x, freqs, attention_mask, Wq, Wk, Wv, Wo, _trace=False, _trace_kwargs=None):
    bf = ml_dtypes.bfloat16
    x = np.asarray(x, dtype=np.float32)
    freqs = np.asarray(freqs, dtype=np.float32)
    mask = np.asarray(attention_mask, dtype=np.float32).reshape(L, L)
    Wq, Wk, Wv, Wo = (np.asarray(w, dtype=np.float32) for w in (Wq, Wk, Wv, Wo))

    causal_ref = np.where(np.tri(L, dtype=bool), 0.0, -1e9).astype(np.float32)
    if not mask.any():
        mask_mode = "zero"
    elif np.array_equal(mask, causal_ref):
        mask_mode = "causal"
    else:
        mask_mode = "general"

    fr, fi = freqs[..., 0], freqs[..., 1]
    cosE = np.repeat(fr, 2, axis=1).T
    sinE = np.repeat(fi, 2, axis=1).T
    cos128 = np.ascontiguousarray(np.concatenate([cosE, cosE], axis=0))
    sin128 = np.ascontiguousarray(np.concatenate([sinE, sinE], axis=0))
    R = np.zeros((128, 128), dtype=np.float32)
    for i in range(64):
        R[2 * i, 2 * i + 1] = -1.0
        R[2 * i + 1, 2 * i] = 1.0
    rT = np.ascontiguousarray(R.T).astype(bf)
    triM = (np.arange(128)[:, None] <= np.arange(128)[None, :]).astype(bf)

    in_maps = []
    for c in range(8):
        b, g = divmod(c, 4)
        rows = slice(DQ * g, DQ * (g + 1))
        def tile_w(wt):
            return np.ascontiguousarray(wt.reshape(8, 128, -1).transpose(1, 0, 2)).astype(bf)
        xt_full = x[b].T
        xt4 = np.ascontiguousarray(
            xt_full.reshape(8, 128, NB, 512).transpose(2, 1, 0, 3)
        ).astype(bf)
        wot = Wo[:, rows].T
        m = {
            "xT": xt4,
            "wqT": tile_w(Wq[rows].T),
            "wkT": tile_w(Wk[rows].T),
            "wvT": tile_w(Wv[rows].T),
            "woT": np.ascontiguousarray(wot.reshape(2, 128, D).transpose(1, 0, 2)).astype(bf),
            "cosT": cos128,
            "sinT": sin128,
            "rT": rT,
            "tri": triM,
            "ones": np.ones((128, 64), dtype=bf),
        }
        if mask_mode == "general":
            m["emT"] = np.ascontiguousarray(np.exp(mask).T)
        in_maps.append(m)

    nc = _get_kernel(mask_mode)
    kw = {}
    if _trace:
        kw = dict(trace=True, trace_kwargs=_trace_kwargs or {})
    res = run_bass_kernel_spmd(nc, in_maps, list(range(8)), **kw)
    out = np.empty((B, L, D), dtype=np.float32)
    for b in range(B):
        acc = res.results[4 * b]["outT"].astype(np.float32)
        for g in range(1, 4):
            acc = acc + res.results[4 * b + g]["outT"].astype(np.float32)
        out[b] = acc.T
    kernel.last_result = res
    return out
